# revision 22
# baseline (speedup 1.0000x reference)
"""Trainium2 Bass kernel for a dense transformer block (LN -> 16-head causal
attention -> residual -> LN -> FFN -> residual) on x:(2, 2048, 1024) fp32.

Sharding: 8 cores, zero collectives. Core c handles batch b=c//4, query chunk
a=c%4 (512 contiguous tokens). Every core recomputes full-sequence K/V for its
batch from a replicated (transposed) copy of x[b]; attention for the 512
queries runs against all 2048 keys with an additive causal mask supplied as
per-core input data, so the compiled program is identical across cores (SPMD).

Everything on-chip runs feature-on-partition ("T-layout"): LayerNorm statistics
are partition reductions done with ones-vector matmuls, the softmax denominator
comes from a ones column appended to V, and per-token stats are broadcast back
across partitions with gpsimd.partition_broadcast. Matmuls run in bf16 with
fp32 PSUM accumulation; both residual adds are carried in fp32.
"""

import numpy as np
import ml_dtypes

import concourse.bass as bass
import concourse.tile as tile
from concourse import bacc, mybir
from concourse import bass_utils
from concourse.bass import ts

P = 128
B, T, C = 2, 2048, 1024
H, D = 16, 64
FF = 4 * C
CC = C // P          # 8 feature chunks
TQ = 512             # queries per core
NSCH = T // P        # 16 key chunks
EPS = 1e-5
NEG = -30000.0
bf16 = ml_dtypes.bfloat16

f32 = mybir.dt.float32
bf = mybir.dt.bfloat16
AF = mybir.ActivationFunctionType
ALU = mybir.AluOpType


def _ln_T(nc, big1, chunked, spsum, x_tile, Tn, out_tile, g_sb, be_sb, eps11,
          ones1, x_is_f32):
    """LayerNorm over the feature dim with activations feature-on-partition.
    x_tile/out_tile: (P, CC, Tn). Stats via ones-matmul partition reduction,
    processed 512 tokens at a time. big1: bufs=1 pool; chunked: bufs>=2."""
    for tch in range(Tn // 512):
        xs = x_tile[:, :, ts(tch, 512)]
        ps = spsum.tile([1, 512], f32, tag="stat")
        if x_is_f32:
            for cc in range(CC):
                xbf = chunked.tile([P, 512], bf, tag="ln_xbf")
                nc.vector.tensor_copy(xbf, xs[:, cc, :])
                nc.tensor.matmul(ps, lhsT=ones1, rhs=xbf,
                                 start=(cc == 0), stop=(cc == CC - 1))
        else:
            for cc in range(CC):
                nc.tensor.matmul(ps, lhsT=ones1, rhs=xs[:, cc, :],
                                 start=(cc == 0), stop=(cc == CC - 1))
        pq = spsum.tile([1, 512], f32, tag="stat")
        for cc in range(CC):
            sq = chunked.tile([P, 512], bf, tag="ln_sq")
            nc.vector.tensor_mul(sq, xs[:, cc, :], xs[:, cc, :])
            nc.tensor.matmul(pq, lhsT=ones1, rhs=sq,
                             start=(cc == 0), stop=(cc == CC - 1))
        m = big1.tile([1, 512], f32, tag="ln_m")
        nc.vector.tensor_scalar_mul(m, ps, 1.0 / C)
        q = big1.tile([1, 512], f32, tag="ln_q")
        nc.vector.tensor_scalar_mul(q, pq, 1.0 / C)
        msq = big1.tile([1, 512], f32, tag="ln_msq")
        nc.vector.tensor_mul(msq, m, m)
        nc.vector.tensor_tensor(q, q, msq, ALU.subtract)  # q := var
        sd = big1.tile([1, 512], f32, tag="ln_sd")
        nc.scalar.activation(sd, q, AF.Sqrt, bias=eps11)
        a_t = big1.tile([1, 512], f32, tag="ln_at")
        nc.vector.reciprocal(a_t, sd)
        b_t = big1.tile([1, 512], f32, tag="ln_bt")
        nc.vector.tensor_mul(b_t, m, a_t)

        a_bc = chunked.tile([P, 512], f32, tag="ln_abc")
        nc.gpsimd.partition_broadcast(a_bc, a_t)
        b_bc = chunked.tile([P, 512], f32, tag="ln_bbc")
        nc.gpsimd.partition_broadcast(b_bc, b_t)
        for cc in range(CC):
            t1 = chunked.tile([P, 512], bf, tag="ln_t1")
            nc.vector.tensor_mul(t1, xs[:, cc, :], a_bc)
            nc.vector.tensor_tensor(t1, t1, b_bc, ALU.subtract)
            nc.vector.tensor_scalar(out_tile[:, cc, ts(tch, 512)], t1,
                                    scalar1=g_sb[:, cc:cc + 1],
                                    scalar2=be_sb[:, cc:cc + 1],
                                    op0=ALU.mult, op1=ALU.add)


FKV = 8 * 512 + 4 * H * 65          # AllGather payload per core (bf16 elems)


def _body(nc, tc, aps, use_ag, bounces):
    (xkvT, xqT, maskT, wq, wk, wv, wo, w1, w2,
     bo_t, b1_t, b2_t, g1_t, be1_t, g2_t, be2_t, outT) = aps

    import contextlib
    ctx = contextlib.ExitStack()
    with ctx:
        # pools that live for the whole kernel (small stuff + psum)
        consts = ctx.enter_context(tc.tile_pool(name="consts", bufs=1))
        small = ctx.enter_context(tc.tile_pool(name="small", bufs=2))
        ppool = ctx.enter_context(tc.tile_pool(name="ppool", bufs=4, space="PSUM"))
        opsum = ctx.enter_context(tc.tile_pool(name="opsum", bufs=2, space="PSUM"))
        spsum = ctx.enter_context(tc.tile_pool(name="spsum", bufs=2, space="PSUM"))

        ones1 = consts.tile([P, 1], bf)
        nc.vector.memset(ones1, 1.0)
        eps11 = consts.tile([1, 1], f32)
        nc.vector.memset(eps11, EPS)

        def load(pool, ap_dram, shape, dtype=f32, tag=None):
            t = pool.tile(list(shape), dtype, tag=tag or ap_dram.name)
            nc.sync.dma_start(t, ap_dram)
            return t

        bo_s = load(consts, bo_t, (P, CC))
        b1_s = load(consts, b1_t, (P, 32))
        b2_s = load(consts, b2_t, (P, CC))
        g1_s = load(consts, g1_t, (P, CC))
        be1_s = load(consts, be1_t, (P, CC))
        g2_s = load(consts, g2_t, (P, CC))
        be2_s = load(consts, be2_t, (P, CC))

        # ---- OT survives until the output projection (opened first: LIFO)
        opool = ctx.enter_context(tc.tile_pool(name="opool", bufs=1))

        # ---- KT/Vr/QT live from the projections to the end of attention
        kvq_ctx = contextlib.ExitStack()
        kvq = kvq_ctx.enter_context(tc.tile_pool(name="kvq", bufs=1))
        KT = kvq.tile([P, 8, T], bf)
        Vr = kvq.tile([P, NSCH, H, 65], bf)
        QT = kvq.tile([P, 8, TQ], bf)

        # ---- hkv/hq live until the end of the QKV projections
        with tc.tile_pool(name="hpool", bufs=1) as hpool:
            hq = hpool.tile([P, CC, TQ], bf)

            if not use_ag:
                hkv = hpool.tile([P, CC, T], bf)
                # LN1 over the full batch sequence (for K/V)
                with tc.tile_pool(name="p_ln1", bufs=1) as p1, \
                     tc.tile_pool(name="p_ln1b", bufs=1) as p1b, \
                     tc.tile_pool(name="p_ln1t", bufs=2) as p1t:
                    xkv_sb = load(p1, xkvT, (P, CC, T), bf)
                    _ln_T(nc, p1b, p1t, spsum, xkv_sb, T, hkv,
                          g1_s, be1_s, eps11, ones1, x_is_f32=False)

            # LN1 over the query slice
            with tc.tile_pool(name="p_ln1q", bufs=1) as p2, \
                 tc.tile_pool(name="p_ln1qb", bufs=1) as p2b, \
                 tc.tile_pool(name="p_ln1qt", bufs=2) as p2t:
                xq_sb = load(p2, xqT, (P, CC, TQ), f32, tag="xq_ln")
                _ln_T(nc, p2b, p2t, spsum, xq_sb, TQ, hq,
                      g1_s, be1_s, eps11, ones1, x_is_f32=True)

            with tc.tile_pool(name="p_w", bufs=1) as pw:
                wq_s = load(pw, wq, (P, CC, 8, P), bf)
                wk_s = load(pw, wk, (P, CC, 8, P), bf)
                wv_s = load(pw, wv, (P, CC, C), bf)

                if use_ag:
                    kv_in, kv_out = bounces
                    # own-chunk K^T (s = this core's 512 tokens)
                    KTo = pw.tile([P, CC, TQ], bf, tag="KTo")
                    for pair in range(8):
                        psum = ppool.tile([P, 512], f32, tag="mm")
                        for cc in range(CC):
                            nc.tensor.matmul(psum, lhsT=wk_s[:, cc, pair, :],
                                             rhs=hq[:, cc, :],
                                             start=(cc == 0), stop=(cc == CC - 1))
                        nc.vector.tensor_copy(KTo[:, pair, :], psum)
                    # own-chunk V rows (4 s-tiles) with ones column
                    Vro = pw.tile([P, 4, H, 65], bf, tag="Vro")
                    nc.vector.memset(Vro[:, :, :, 64:65], 1.0)
                    for st in range(4):
                        for half in range(2):
                            psum = ppool.tile([P, 512], f32, tag="mm")
                            for cc in range(CC):
                                nc.tensor.matmul(psum, lhsT=hq[:, cc, ts(st, P)],
                                                 rhs=wv_s[:, cc, ts(half, 512)],
                                                 start=(cc == 0), stop=(cc == CC - 1))
                            nc.vector.tensor_copy(
                                Vro[:, st, half * 8:(half + 1) * 8, 0:64],
                                psum.rearrange("p (h d) -> p h d", d=64))
                    # bounce out, AllGather within the 4-core batch group,
                    # then scatter the gathered chunks into KT / Vr
                    nc.sync.dma_start(
                        kv_in.ap()[:, 0:4096].rearrange("p (a b) -> p a b", a=CC),
                        KTo)
                    nc.sync.dma_start(
                        kv_in.ap()[:, 4096:FKV].rearrange(
                            "p (a h e) -> p a h e", a=4, h=H), Vro)
                    nc.gpsimd.collective_compute(
                        "AllGather",
                        mybir.AluOpType.bypass,
                        replica_groups=[[0, 1, 2, 3], [4, 5, 6, 7]],
                        ins=[kv_in.ap().opt()],
                        outs=[kv_out.ap().opt()],
                    )
                    for r in range(4):
                        blk = kv_out.ap()[r * P:(r + 1) * P, :]
                        nc.sync.dma_start(
                            KT[:, :, ts(r, 512)],
                            blk[:, 0:4096].rearrange("p (a b) -> p a b", a=CC))
                        nc.sync.dma_start(
                            Vr[:, 4 * r:4 * (r + 1), :, :],
                            blk[:, 4096:FKV].rearrange(
                                "p (a h e) -> p a h e", a=4, h=H))
                else:
                    for pair in range(8):
                        for sch in range(4):
                            psum = ppool.tile([P, 512], f32, tag="mm")
                            for cc in range(CC):
                                nc.tensor.matmul(psum, lhsT=wk_s[:, cc, pair, :],
                                                 rhs=hkv[:, cc, ts(sch, 512)],
                                                 start=(cc == 0), stop=(cc == CC - 1))
                            nc.vector.tensor_copy(KT[:, pair, ts(sch, 512)], psum)

                    nc.vector.memset(Vr[:, :, :, 64:65], 1.0)
                    for st in range(NSCH):
                        for half in range(2):
                            psum = ppool.tile([P, 512], f32, tag="mm")
                            for cc in range(CC):
                                nc.tensor.matmul(psum, lhsT=hkv[:, cc, ts(st, P)],
                                                 rhs=wv_s[:, cc, ts(half, 512)],
                                                 start=(cc == 0), stop=(cc == CC - 1))
                            nc.vector.tensor_copy(
                                Vr[:, st, half * 8:(half + 1) * 8, 0:64],
                                psum.rearrange("p (h d) -> p h d", d=64))

                for pair in range(8):
                    psum = ppool.tile([P, 512], f32, tag="mm")
                    for cc in range(CC):
                        nc.tensor.matmul(psum, lhsT=wq_s[:, cc, pair, :],
                                         rhs=hq[:, cc, :],
                                         start=(cc == 0), stop=(cc == CC - 1))
                    nc.vector.tensor_scalar_mul(QT[:, pair, :], psum,
                                                float(C) ** -0.5)

        # ---- attention
        OT = opool.tile([P, 8, TQ], bf)
        with tc.tile_pool(name="p_att", bufs=1) as pa, \
             tc.tile_pool(name="p_attt", bufs=3) as pat:
            mask_sb = load(pa, maskT, (P, NSCH, TQ), bf)
            for h in range(H):
                pair, half = h // 2, h % 2
                hp = slice(64 * half, 64 * half + 64)
                ops = opsum.tile([P, 512], f32, tag="av")
                for sch in range(NSCH):
                    sps = ppool.tile([P, 512], f32, tag="mm")
                    nc.tensor.matmul(sps, lhsT=KT[hp, pair, ts(sch, P)],
                                     rhs=QT[hp, pair, :], start=True, stop=True)
                    sm = pat.tile([P, TQ], bf, tag="sm")
                    nc.vector.tensor_tensor(sm, sps, mask_sb[:, sch, :], ALU.add)
                    e = pat.tile([P, TQ], bf, tag="e")
                    nc.scalar.activation(e, sm, AF.Exp)
                    nc.tensor.matmul(ops[0:65, :], lhsT=Vr[:, sch, h, :], rhs=e,
                                     start=(sch == 0), stop=(sch == NSCH - 1))
                zr = small.tile([1, TQ], f32, tag="zr")
                nc.vector.reciprocal(zr, ops[64:65, :])
                zb = pat.tile([64, TQ], f32, tag="zb")
                nc.gpsimd.partition_broadcast(zb, zr)
                nc.vector.tensor_mul(OT[hp, pair, :], ops[0:64, :], zb)
        kvq_ctx.close()

        # ---- output projection + bias + residual; LN2; FFN
        with tc.tile_pool(name="p_ffn", bufs=1) as pf, \
             tc.tile_pool(name="p_ffnt", bufs=2) as pft, \
             tc.tile_pool(name="p_wstream", bufs=3) as pws:
            xq_sb = load(pf, xqT, (P, CC, TQ), f32, tag="xq_res")
            wo_s = load(pf, wo, (P, CC, 8, P), bf)
            y1 = pf.tile([P, CC, TQ], f32)
            for mo in range(CC):
                psum = ppool.tile([P, 512], f32, tag="mm")
                for cc in range(CC):
                    nc.tensor.matmul(psum, lhsT=wo_s[:, cc, mo, :],
                                     rhs=OT[:, cc, :],
                                     start=(cc == 0), stop=(cc == CC - 1))
                t = pft.tile([P, TQ], f32, tag="res")
                nc.vector.tensor_scalar_add(t, psum, bo_s[:, mo:mo + 1])
                nc.vector.tensor_tensor(y1[:, mo, :], t, xq_sb[:, mo, :], ALU.add)

            h2 = pf.tile([P, CC, TQ], bf)
            with tc.tile_pool(name="p_ln2b", bufs=1) as pl2b:
                _ln_T(nc, pl2b, pft, spsum, y1, TQ, h2,
                      g2_s, be2_s, eps11, ones1, x_is_f32=True)

            zT = pf.tile([P, 32, TQ], bf)
            for m in range(32):
                w1b = pws.tile([P, CC, P], bf, tag="w1")
                nc.sync.dma_start(w1b, w1[m])
                psum = ppool.tile([P, 512], f32, tag="mm")
                for cc in range(CC):
                    nc.tensor.matmul(psum, lhsT=w1b[:, cc, :], rhs=h2[:, cc, :],
                                     start=(cc == 0), stop=(cc == CC - 1))
                nc.scalar.activation(zT[:, m, :], psum, AF.Relu,
                                     bias=b1_s[:, m:m + 1])

            for mo in range(CC):
                w2b = pws.tile([P, 32, P], bf, tag="w2")
                nc.sync.dma_start(w2b, w2[mo])
                psum = ppool.tile([P, 512], f32, tag="mm")
                for ff in range(32):
                    nc.tensor.matmul(psum, lhsT=w2b[:, ff, :], rhs=zT[:, ff, :],
                                     start=(ff == 0), stop=(ff == 31))
                t = pft.tile([P, TQ], f32, tag="res")
                nc.vector.tensor_scalar_add(t, psum, b2_s[:, mo:mo + 1])
                ot = pft.tile([P, TQ], f32, tag="ot")
                nc.vector.tensor_tensor(ot, t, y1[:, mo, :], ALU.add)
                nc.sync.dma_start(outT[:, mo, :], ot)


_NC_CACHE = {}
USE_AG = False


def build_nc(reps=1, use_ag=None):
    global _NC_CACHE
    if use_ag is None:
        use_ag = USE_AG
    key = (reps, use_ag)
    if key in _NC_CACHE:
        return _NC_CACHE[key]
    nc = bacc.Bacc("TRN2", target_bir_lowering=False, debug=False,
                   enable_asserts=False, num_devices=8)

    def dram(name, shape, dtype, kind="ExternalInput"):
        return nc.dram_tensor(name, shape, dtype, kind=kind).ap()

    aps = (
        dram("xkvT", (P, CC, T), bf) if not use_ag else None,
        dram("xqT", (P, CC, TQ), f32),
        dram("maskT", (P, NSCH, TQ), bf),
        dram("wq", (P, CC, 8, P), bf),
        dram("wk", (P, CC, 8, P), bf),
        dram("wv", (P, CC, C), bf),
        dram("wo", (P, CC, 8, P), bf),
        dram("w1", (32, P, CC, P), bf),
        dram("w2", (CC, P, 32, P), bf),
        dram("bo_t", (P, CC), f32),
        dram("b1_t", (P, 32), f32),
        dram("b2_t", (P, CC), f32),
        dram("g1_t", (P, CC), f32),
        dram("be1_t", (P, CC), f32),
        dram("g2_t", (P, CC), f32),
        dram("be2_t", (P, CC), f32),
        dram("outT", (P, CC, TQ), f32, kind="ExternalOutput"),
    )
    bounces = None
    if use_ag:
        bounces = []
        for i in range(reps):
            kv_in = nc.dram_tensor(f"kv_in{i}", (P, FKV), bf)
            kv_out = nc.dram_tensor(f"kv_out{i}", (4 * P, FKV), bf)
            bounces.append((kv_in, kv_out))
    with tile.TileContext(nc) as tc:
        for i in range(reps):
            _body(nc, tc, aps, use_ag, bounces[i] if use_ag else None)
    nc.compile()
    _NC_CACHE[key] = nc
    return nc


def _tile_lhst(w):  # (C, C) -> (P, cc, pair/mo, 128)
    return np.ascontiguousarray(
        w.reshape(CC, P, 8, P).transpose(1, 0, 2, 3)).astype(bf16)


def make_in_maps(inputs, use_ag=None):
    """Build the 8 per-core input dicts from the full problem inputs."""
    if use_ag is None:
        use_ag = USE_AG
    x = np.asarray(inputs["x"], np.float32)
    Wq = np.asarray(inputs["Wq"], np.float32)
    Wk = np.asarray(inputs["Wk"], np.float32)
    Wv = np.asarray(inputs["Wv"], np.float32)
    Wo = np.asarray(inputs["Wo"], np.float32)
    W1 = np.asarray(inputs["W1"], np.float32)
    W2 = np.asarray(inputs["W2"], np.float32)

    wq_flat = np.ascontiguousarray(Wq.transpose(1, 0, 2)).reshape(C, C)
    wk_flat = np.ascontiguousarray(Wk.transpose(1, 0, 2)).reshape(C, C)
    wv_flat = np.ascontiguousarray(Wv.transpose(1, 0, 2)).reshape(C, C)

    shared = {
        "wq": _tile_lhst(wq_flat),
        "wk": _tile_lhst(wk_flat),
        "wv": np.ascontiguousarray(
            wv_flat.reshape(CC, P, C).transpose(1, 0, 2)).astype(bf16),
        "wo": _tile_lhst(Wo),
        "w1": np.ascontiguousarray(
            W1.reshape(CC, P, 32, P).transpose(2, 1, 0, 3)).astype(bf16),
        "w2": np.ascontiguousarray(
            W2.reshape(32, P, CC, P).transpose(2, 1, 0, 3)).astype(bf16),
        "bo_t": np.ascontiguousarray(
            np.asarray(inputs["bo"], np.float32).reshape(CC, P).T),
        "b1_t": np.ascontiguousarray(
            np.asarray(inputs["b1"], np.float32).reshape(32, P).T),
        "b2_t": np.ascontiguousarray(
            np.asarray(inputs["b2"], np.float32).reshape(CC, P).T),
        "g1_t": np.ascontiguousarray(
            np.asarray(inputs["g1"], np.float32).reshape(CC, P).T),
        "be1_t": np.ascontiguousarray(
            np.asarray(inputs["be1"], np.float32).reshape(CC, P).T),
        "g2_t": np.ascontiguousarray(
            np.asarray(inputs["g2"], np.float32).reshape(CC, P).T),
        "be2_t": np.ascontiguousarray(
            np.asarray(inputs["be2"], np.float32).reshape(CC, P).T),
    }

    s_idx = np.arange(T)
    in_maps = []
    for c in range(8):
        b, a = c // 4, c % 4
        q0 = TQ * a
        xbT = np.ascontiguousarray(x[b].T)                       # (C, T)
        xkvT = xbT.reshape(CC, P, T).transpose(1, 0, 2).astype(bf16)
        xqT = np.ascontiguousarray(
            xbT[:, q0:q0 + TQ].reshape(CC, P, TQ).transpose(1, 0, 2))
        mask = np.where(s_idx[:, None] <= (q0 + np.arange(TQ))[None, :],
                        np.float32(0.0), np.float32(NEG))
        maskT = mask.reshape(NSCH, P, TQ).transpose(1, 0, 2).astype(bf16)
        m = {
            "xqT": xqT.astype(np.float32),
            "maskT": np.ascontiguousarray(maskT),
            **shared,
        }
        if not use_ag:
            m["xkvT"] = np.ascontiguousarray(xkvT)
        in_maps.append(m)
    return in_maps


def assemble_output(core_outs):
    """core_outs: list of 8 dicts with 'outT' (P, CC, TQ) fp32."""
    out = np.zeros((B, T, C), np.float32)
    for c in range(8):
        b, a = c // 4, c % 4
        y2 = core_outs[c]["outT"].transpose(1, 0, 2).reshape(C, TQ)  # (C, TQ)
        out[b, TQ * a:TQ * (a + 1), :] = y2.T
    return out


def kernel(**inputs) -> np.ndarray:
    nc = build_nc()
    in_maps = make_in_maps(inputs)
    res = bass_utils.run_bass_kernel_spmd(nc, in_maps, core_ids=list(range(8)))
    return assemble_output(res.results)


if __name__ == "__main__":
    import reference
    inputs = {k: np.asarray(v) for k, v in reference.setup_inputs().items()}
    expected = np.asarray(reference.reference(**inputs))
    actual = kernel(**inputs)
    err = np.abs(actual - expected)
    print("absmax err:", err.max(), "scale:", np.abs(expected).max())
    print("rel fro:", np.linalg.norm(actual - expected) / np.linalg.norm(expected))


# revision 23
# speedup vs baseline: 120.6307x; 120.6307x over previous
"""Trainium2 Bass kernel for a dense transformer block (LN -> 16-head causal
attention -> residual -> LN -> FFN -> residual) on x:(2, 2048, 1024) fp32.

Sharding: 8 cores, zero collectives. Core c handles batch b=c//4, query chunk
a=c%4 (512 contiguous tokens). Every core recomputes full-sequence K/V for its
batch from a replicated (transposed) copy of x[b]; attention for the 512
queries runs against all 2048 keys with an additive causal mask supplied as
per-core input data, so the compiled program is identical across cores (SPMD).

Everything on-chip runs feature-on-partition ("T-layout"): LayerNorm statistics
are partition reductions done with ones-vector matmuls, the softmax denominator
comes from a ones column appended to V, and per-token stats are broadcast back
across partitions with gpsimd.partition_broadcast. Matmuls run in bf16 with
fp32 PSUM accumulation; both residual adds are carried in fp32.
"""

import numpy as np
import ml_dtypes

import concourse.bass as bass
import concourse.tile as tile
from concourse import bacc, mybir
from concourse import bass_utils
from concourse.bass import ts

P = 128
B, T, C = 2, 2048, 1024
H, D = 16, 64
FF = 4 * C
CC = C // P          # 8 feature chunks
TQ = 512             # queries per core
NSCH = T // P        # 16 key chunks
EPS = 1e-5
NEG = -30000.0
bf16 = ml_dtypes.bfloat16

f32 = mybir.dt.float32
bf = mybir.dt.bfloat16
AF = mybir.ActivationFunctionType
ALU = mybir.AluOpType


def _ln_T(nc, big1, chunked, spsum, x_tile, Tn, out_tile, g_sb, be_sb, eps11,
          ones1, x_is_f32):
    """LayerNorm over the feature dim with activations feature-on-partition.
    x_tile/out_tile: (P, CC, Tn). Stats via ones-matmul partition reduction,
    processed 512 tokens at a time. big1: bufs=1 pool; chunked: bufs>=2."""
    for tch in range(Tn // 512):
        xs = x_tile[:, :, ts(tch, 512)]
        ps = spsum.tile([1, 512], f32, tag="stat")
        if x_is_f32:
            for cc in range(CC):
                xbf = chunked.tile([P, 512], bf, tag="ln_xbf")
                nc.vector.tensor_copy(xbf, xs[:, cc, :])
                nc.tensor.matmul(ps, lhsT=ones1, rhs=xbf,
                                 start=(cc == 0), stop=(cc == CC - 1))
        else:
            for cc in range(CC):
                nc.tensor.matmul(ps, lhsT=ones1, rhs=xs[:, cc, :],
                                 start=(cc == 0), stop=(cc == CC - 1))
        pq = spsum.tile([1, 512], f32, tag="stat")
        for cc in range(CC):
            sq = chunked.tile([P, 512], bf, tag="ln_sq")
            nc.vector.tensor_mul(sq, xs[:, cc, :], xs[:, cc, :])
            nc.tensor.matmul(pq, lhsT=ones1, rhs=sq,
                             start=(cc == 0), stop=(cc == CC - 1))
        m = big1.tile([1, 512], f32, tag="ln_m")
        nc.vector.tensor_scalar_mul(m, ps, 1.0 / C)
        q = big1.tile([1, 512], f32, tag="ln_q")
        nc.vector.tensor_scalar_mul(q, pq, 1.0 / C)
        msq = big1.tile([1, 512], f32, tag="ln_msq")
        nc.vector.tensor_mul(msq, m, m)
        nc.vector.tensor_tensor(q, q, msq, ALU.subtract)  # q := var
        sd = big1.tile([1, 512], f32, tag="ln_sd")
        nc.scalar.activation(sd, q, AF.Sqrt, bias=eps11)
        a_t = big1.tile([1, 512], f32, tag="ln_at")
        nc.vector.reciprocal(a_t, sd)
        b_t = big1.tile([1, 512], f32, tag="ln_bt")
        nc.vector.tensor_mul(b_t, m, a_t)

        a_bc = chunked.tile([P, 512], f32, tag="ln_abc")
        nc.gpsimd.partition_broadcast(a_bc, a_t)
        b_bc = chunked.tile([P, 512], f32, tag="ln_bbc")
        nc.gpsimd.partition_broadcast(b_bc, b_t)
        for cc in range(CC):
            t1 = chunked.tile([P, 512], bf, tag="ln_t1")
            nc.vector.tensor_mul(t1, xs[:, cc, :], a_bc)
            nc.vector.tensor_tensor(t1, t1, b_bc, ALU.subtract)
            nc.vector.tensor_scalar(out_tile[:, cc, ts(tch, 512)], t1,
                                    scalar1=g_sb[:, cc:cc + 1],
                                    scalar2=be_sb[:, cc:cc + 1],
                                    op0=ALU.mult, op1=ALU.add)


FKV = 8 * 512 + 4 * H * 65          # AllGather payload per core (bf16 elems)


def _body(nc, tc, aps, use_ag, bounces):
    (xkvT, xqT, maskT, wq, wk, wv, wo, w1, w2,
     bo_t, b1_t, b2_t, g1_t, be1_t, g2_t, be2_t, outT) = aps

    import contextlib
    ctx = contextlib.ExitStack()
    with ctx:
        # pools that live for the whole kernel (small stuff + psum)
        consts = ctx.enter_context(tc.tile_pool(name="consts", bufs=1))
        small = ctx.enter_context(tc.tile_pool(name="small", bufs=2))
        ppool = ctx.enter_context(tc.tile_pool(name="ppool", bufs=4, space="PSUM"))
        opsum = ctx.enter_context(tc.tile_pool(name="opsum", bufs=2, space="PSUM"))
        spsum = ctx.enter_context(tc.tile_pool(name="spsum", bufs=2, space="PSUM"))

        ones1 = consts.tile([P, 1], bf)
        nc.vector.memset(ones1, 1.0)
        eps11 = consts.tile([1, 1], f32)
        nc.vector.memset(eps11, EPS)

        def load(pool, ap_dram, shape, dtype=f32, tag=None):
            t = pool.tile(list(shape), dtype, tag=tag or ap_dram.name)
            nc.sync.dma_start(t, ap_dram)
            return t

        bo_s = load(consts, bo_t, (P, CC))
        b1_s = load(consts, b1_t, (P, 32))
        b2_s = load(consts, b2_t, (P, CC))
        g1_s = load(consts, g1_t, (P, CC))
        be1_s = load(consts, be1_t, (P, CC))
        g2_s = load(consts, g2_t, (P, CC))
        be2_s = load(consts, be2_t, (P, CC))

        # ---- OT survives until the output projection (opened first: LIFO)
        opool = ctx.enter_context(tc.tile_pool(name="opool", bufs=1))

        # ---- KT/Vr/QT live from the projections to the end of attention
        kvq_ctx = contextlib.ExitStack()
        kvq = kvq_ctx.enter_context(tc.tile_pool(name="kvq", bufs=1))
        KT = kvq.tile([P, 8, T], bf)
        Vr = kvq.tile([P, NSCH, H, 65], bf)
        QT = kvq.tile([P, 8, TQ], bf)

        # ---- hkv/hq live until the end of the QKV projections
        with tc.tile_pool(name="hpool", bufs=1) as hpool:
            hq = hpool.tile([P, CC, TQ], bf)

            if not use_ag:
                hkv = hpool.tile([P, CC, T], bf)
                # LN1 over the full batch sequence (for K/V)
                with tc.tile_pool(name="p_ln1", bufs=1) as p1, \
                     tc.tile_pool(name="p_ln1b", bufs=1) as p1b, \
                     tc.tile_pool(name="p_ln1t", bufs=2) as p1t:
                    xkv_sb = load(p1, xkvT, (P, CC, T), bf)
                    _ln_T(nc, p1b, p1t, spsum, xkv_sb, T, hkv,
                          g1_s, be1_s, eps11, ones1, x_is_f32=False)

            # LN1 over the query slice
            with tc.tile_pool(name="p_ln1q", bufs=1) as p2, \
                 tc.tile_pool(name="p_ln1qb", bufs=1) as p2b, \
                 tc.tile_pool(name="p_ln1qt", bufs=2) as p2t:
                xq_sb = load(p2, xqT, (P, CC, TQ), f32, tag="xq_ln")
                _ln_T(nc, p2b, p2t, spsum, xq_sb, TQ, hq,
                      g1_s, be1_s, eps11, ones1, x_is_f32=True)

            with tc.tile_pool(name="p_w", bufs=1) as pw:
                wq_s = load(pw, wq, (P, CC, 8, P), bf)
                wk_s = load(pw, wk, (P, CC, 8, P), bf)
                wv_s = load(pw, wv, (P, CC, C), bf)

                if use_ag:
                    kv_in, kv_out = bounces
                    # own-chunk K^T (s = this core's 512 tokens)
                    KTo = pw.tile([P, CC, TQ], bf, tag="KTo")
                    for pair in range(8):
                        psum = ppool.tile([P, 512], f32, tag="mm")
                        for cc in range(CC):
                            nc.tensor.matmul(psum, lhsT=wk_s[:, cc, pair, :],
                                             rhs=hq[:, cc, :],
                                             start=(cc == 0), stop=(cc == CC - 1))
                        nc.vector.tensor_copy(KTo[:, pair, :], psum)
                    # own-chunk V rows (4 s-tiles) with ones column
                    Vro = pw.tile([P, 4, H, 65], bf, tag="Vro")
                    nc.vector.memset(Vro[:, :, :, 64:65], 1.0)
                    for st in range(4):
                        for half in range(2):
                            psum = ppool.tile([P, 512], f32, tag="mm")
                            for cc in range(CC):
                                nc.tensor.matmul(psum, lhsT=hq[:, cc, ts(st, P)],
                                                 rhs=wv_s[:, cc, ts(half, 512)],
                                                 start=(cc == 0), stop=(cc == CC - 1))
                            nc.vector.tensor_copy(
                                Vro[:, st, half * 8:(half + 1) * 8, 0:64],
                                psum.rearrange("p (h d) -> p h d", d=64))
                    # bounce out, AllGather within the 4-core batch group,
                    # then scatter the gathered chunks into KT / Vr
                    nc.sync.dma_start(
                        kv_in.ap()[:, 0:4096].rearrange("p (a b) -> p a b", a=CC),
                        KTo)
                    nc.sync.dma_start(
                        kv_in.ap()[:, 4096:FKV].rearrange(
                            "p (a h e) -> p a h e", a=4, h=H), Vro)
                    nc.gpsimd.collective_compute(
                        "AllGather",
                        mybir.AluOpType.bypass,
                        replica_groups=[[0, 1, 2, 3], [4, 5, 6, 7]],
                        ins=[kv_in.ap().opt()],
                        outs=[kv_out.ap().opt()],
                    )
                    for r in range(4):
                        blk = kv_out.ap()[r * P:(r + 1) * P, :]
                        nc.sync.dma_start(
                            KT[:, :, ts(r, 512)],
                            blk[:, 0:4096].rearrange("p (a b) -> p a b", a=CC))
                        nc.sync.dma_start(
                            Vr[:, 4 * r:4 * (r + 1), :, :],
                            blk[:, 4096:FKV].rearrange(
                                "p (a h e) -> p a h e", a=4, h=H))
                else:
                    for pair in range(8):
                        for sch in range(4):
                            psum = ppool.tile([P, 512], f32, tag="mm")
                            for cc in range(CC):
                                nc.tensor.matmul(psum, lhsT=wk_s[:, cc, pair, :],
                                                 rhs=hkv[:, cc, ts(sch, 512)],
                                                 start=(cc == 0), stop=(cc == CC - 1))
                            nc.vector.tensor_copy(KT[:, pair, ts(sch, 512)], psum)

                    nc.vector.memset(Vr[:, :, :, 64:65], 1.0)
                    for st in range(NSCH):
                        for half in range(2):
                            psum = ppool.tile([P, 512], f32, tag="mm")
                            for cc in range(CC):
                                nc.tensor.matmul(psum, lhsT=hkv[:, cc, ts(st, P)],
                                                 rhs=wv_s[:, cc, ts(half, 512)],
                                                 start=(cc == 0), stop=(cc == CC - 1))
                            nc.vector.tensor_copy(
                                Vr[:, st, half * 8:(half + 1) * 8, 0:64],
                                psum.rearrange("p (h d) -> p h d", d=64))

                for pair in range(8):
                    psum = ppool.tile([P, 512], f32, tag="mm")
                    for cc in range(CC):
                        nc.tensor.matmul(psum, lhsT=wq_s[:, cc, pair, :],
                                         rhs=hq[:, cc, :],
                                         start=(cc == 0), stop=(cc == CC - 1))
                    nc.vector.tensor_scalar_mul(QT[:, pair, :], psum,
                                                float(C) ** -0.5)

        # ---- attention
        OT = opool.tile([P, 8, TQ], bf)
        with tc.tile_pool(name="p_att", bufs=1) as pa, \
             tc.tile_pool(name="p_attt", bufs=4) as pat:
            mask_sb = load(pa, maskT, (P, NSCH, TQ), bf)
            for h in range(H):
                pair, half = h // 2, h % 2
                hp = slice(64 * half, 64 * half + 64)
                ops = opsum.tile([P, 512], f32, tag="av")
                for sch in range(NSCH):
                    sps = ppool.tile([P, 512], f32, tag="mm")
                    nc.tensor.matmul(sps, lhsT=KT[hp, pair, ts(sch, P)],
                                     rhs=QT[hp, pair, :], start=True, stop=True)
                    sm = pat.tile([P, TQ], bf, tag="sm")
                    nc.vector.tensor_tensor(sm, sps, mask_sb[:, sch, :], ALU.add)
                    e = pat.tile([P, TQ], bf, tag="e")
                    nc.scalar.activation(e, sm, AF.Exp)
                    nc.tensor.matmul(ops[0:65, :], lhsT=Vr[:, sch, h, :], rhs=e,
                                     start=(sch == 0), stop=(sch == NSCH - 1))
                zr = small.tile([1, TQ], f32, tag="zr")
                nc.vector.reciprocal(zr, ops[64:65, :])
                zb = pat.tile([64, TQ], f32, tag="zb")
                nc.gpsimd.partition_broadcast(zb, zr)
                nc.vector.tensor_mul(OT[hp, pair, :], ops[0:64, :], zb)
        kvq_ctx.close()

        # ---- output projection + bias + residual; LN2; FFN
        with tc.tile_pool(name="p_ffn", bufs=1) as pf, \
             tc.tile_pool(name="p_ffnt", bufs=2) as pft, \
             tc.tile_pool(name="p_wstream", bufs=3) as pws:
            xq_sb = load(pf, xqT, (P, CC, TQ), f32, tag="xq_res")
            wo_s = load(pf, wo, (P, CC, 8, P), bf)
            y1 = pf.tile([P, CC, TQ], f32)
            for mo in range(CC):
                psum = ppool.tile([P, 512], f32, tag="mm")
                for cc in range(CC):
                    nc.tensor.matmul(psum, lhsT=wo_s[:, cc, mo, :],
                                     rhs=OT[:, cc, :],
                                     start=(cc == 0), stop=(cc == CC - 1))
                t = pft.tile([P, TQ], f32, tag="res")
                nc.vector.tensor_scalar_add(t, psum, bo_s[:, mo:mo + 1])
                nc.vector.tensor_tensor(y1[:, mo, :], t, xq_sb[:, mo, :], ALU.add)

            h2 = pf.tile([P, CC, TQ], bf)
            with tc.tile_pool(name="p_ln2b", bufs=1) as pl2b:
                _ln_T(nc, pl2b, pft, spsum, y1, TQ, h2,
                      g2_s, be2_s, eps11, ones1, x_is_f32=True)

            zT = pf.tile([P, 32, TQ], bf)
            for m in range(32):
                w1b = pws.tile([P, CC, P], bf, tag="w1")
                nc.sync.dma_start(w1b, w1[m])
                psum = ppool.tile([P, 512], f32, tag="mm")
                for cc in range(CC):
                    nc.tensor.matmul(psum, lhsT=w1b[:, cc, :], rhs=h2[:, cc, :],
                                     start=(cc == 0), stop=(cc == CC - 1))
                nc.scalar.activation(zT[:, m, :], psum, AF.Relu,
                                     bias=b1_s[:, m:m + 1])

            for mo in range(CC):
                w2b = pws.tile([P, 32, P], bf, tag="w2")
                nc.sync.dma_start(w2b, w2[mo])
                psum = ppool.tile([P, 512], f32, tag="mm")
                for ff in range(32):
                    nc.tensor.matmul(psum, lhsT=w2b[:, ff, :], rhs=zT[:, ff, :],
                                     start=(ff == 0), stop=(ff == 31))
                t = pft.tile([P, TQ], f32, tag="res")
                nc.vector.tensor_scalar_add(t, psum, b2_s[:, mo:mo + 1])
                ot = pft.tile([P, TQ], f32, tag="ot")
                nc.vector.tensor_tensor(ot, t, y1[:, mo, :], ALU.add)
                nc.sync.dma_start(outT[:, mo, :], ot)


_NC_CACHE = {}
USE_AG = False


def build_nc(reps=1, use_ag=None):
    global _NC_CACHE
    if use_ag is None:
        use_ag = USE_AG
    key = (reps, use_ag)
    if key in _NC_CACHE:
        return _NC_CACHE[key]
    nc = bacc.Bacc("TRN2", target_bir_lowering=False, debug=False,
                   enable_asserts=False, num_devices=8)

    def dram(name, shape, dtype, kind="ExternalInput"):
        return nc.dram_tensor(name, shape, dtype, kind=kind).ap()

    aps = (
        dram("xkvT", (P, CC, T), bf) if not use_ag else None,
        dram("xqT", (P, CC, TQ), f32),
        dram("maskT", (P, NSCH, TQ), bf),
        dram("wq", (P, CC, 8, P), bf),
        dram("wk", (P, CC, 8, P), bf),
        dram("wv", (P, CC, C), bf),
        dram("wo", (P, CC, 8, P), bf),
        dram("w1", (32, P, CC, P), bf),
        dram("w2", (CC, P, 32, P), bf),
        dram("bo_t", (P, CC), f32),
        dram("b1_t", (P, 32), f32),
        dram("b2_t", (P, CC), f32),
        dram("g1_t", (P, CC), f32),
        dram("be1_t", (P, CC), f32),
        dram("g2_t", (P, CC), f32),
        dram("be2_t", (P, CC), f32),
        dram("outT", (P, CC, TQ), f32, kind="ExternalOutput"),
    )
    bounces = None
    if use_ag:
        bounces = []
        for i in range(reps):
            kv_in = nc.dram_tensor(f"kv_in{i}", (P, FKV), bf)
            kv_out = nc.dram_tensor(f"kv_out{i}", (4 * P, FKV), bf)
            bounces.append((kv_in, kv_out))
    with tile.TileContext(nc) as tc:
        for i in range(reps):
            _body(nc, tc, aps, use_ag, bounces[i] if use_ag else None)
    nc.compile()
    _NC_CACHE[key] = nc
    return nc


def _tile_lhst(w):  # (C, C) -> (P, cc, pair/mo, 128)
    return np.ascontiguousarray(
        w.reshape(CC, P, 8, P).transpose(1, 0, 2, 3)).astype(bf16)


def make_in_maps(inputs, use_ag=None):
    """Build the 8 per-core input dicts from the full problem inputs."""
    if use_ag is None:
        use_ag = USE_AG
    x = np.asarray(inputs["x"], np.float32)
    Wq = np.asarray(inputs["Wq"], np.float32)
    Wk = np.asarray(inputs["Wk"], np.float32)
    Wv = np.asarray(inputs["Wv"], np.float32)
    Wo = np.asarray(inputs["Wo"], np.float32)
    W1 = np.asarray(inputs["W1"], np.float32)
    W2 = np.asarray(inputs["W2"], np.float32)

    wq_flat = np.ascontiguousarray(Wq.transpose(1, 0, 2)).reshape(C, C)
    wk_flat = np.ascontiguousarray(Wk.transpose(1, 0, 2)).reshape(C, C)
    wv_flat = np.ascontiguousarray(Wv.transpose(1, 0, 2)).reshape(C, C)

    shared = {
        "wq": _tile_lhst(wq_flat),
        "wk": _tile_lhst(wk_flat),
        "wv": np.ascontiguousarray(
            wv_flat.reshape(CC, P, C).transpose(1, 0, 2)).astype(bf16),
        "wo": _tile_lhst(Wo),
        "w1": np.ascontiguousarray(
            W1.reshape(CC, P, 32, P).transpose(2, 1, 0, 3)).astype(bf16),
        "w2": np.ascontiguousarray(
            W2.reshape(32, P, CC, P).transpose(2, 1, 0, 3)).astype(bf16),
        "bo_t": np.ascontiguousarray(
            np.asarray(inputs["bo"], np.float32).reshape(CC, P).T),
        "b1_t": np.ascontiguousarray(
            np.asarray(inputs["b1"], np.float32).reshape(32, P).T),
        "b2_t": np.ascontiguousarray(
            np.asarray(inputs["b2"], np.float32).reshape(CC, P).T),
        "g1_t": np.ascontiguousarray(
            np.asarray(inputs["g1"], np.float32).reshape(CC, P).T),
        "be1_t": np.ascontiguousarray(
            np.asarray(inputs["be1"], np.float32).reshape(CC, P).T),
        "g2_t": np.ascontiguousarray(
            np.asarray(inputs["g2"], np.float32).reshape(CC, P).T),
        "be2_t": np.ascontiguousarray(
            np.asarray(inputs["be2"], np.float32).reshape(CC, P).T),
    }

    s_idx = np.arange(T)
    in_maps = []
    for c in range(8):
        b, a = c // 4, c % 4
        q0 = TQ * a
        xbT = np.ascontiguousarray(x[b].T)                       # (C, T)
        xkvT = xbT.reshape(CC, P, T).transpose(1, 0, 2).astype(bf16)
        xqT = np.ascontiguousarray(
            xbT[:, q0:q0 + TQ].reshape(CC, P, TQ).transpose(1, 0, 2))
        mask = np.where(s_idx[:, None] <= (q0 + np.arange(TQ))[None, :],
                        np.float32(0.0), np.float32(NEG))
        maskT = mask.reshape(NSCH, P, TQ).transpose(1, 0, 2).astype(bf16)
        m = {
            "xqT": xqT.astype(np.float32),
            "maskT": np.ascontiguousarray(maskT),
            **shared,
        }
        if not use_ag:
            m["xkvT"] = np.ascontiguousarray(xkvT)
        in_maps.append(m)
    return in_maps


def assemble_output(core_outs):
    """core_outs: list of 8 dicts with 'outT' (P, CC, TQ) fp32."""
    out = np.zeros((B, T, C), np.float32)
    for c in range(8):
        b, a = c // 4, c % 4
        y2 = core_outs[c]["outT"].transpose(1, 0, 2).reshape(C, TQ)  # (C, TQ)
        out[b, TQ * a:TQ * (a + 1), :] = y2.T
    return out


def kernel(**inputs) -> np.ndarray:
    nc = build_nc()
    in_maps = make_in_maps(inputs)
    res = bass_utils.run_bass_kernel_spmd(nc, in_maps, core_ids=list(range(8)))
    return assemble_output(res.results)


if __name__ == "__main__":
    import reference
    inputs = {k: np.asarray(v) for k, v in reference.setup_inputs().items()}
    expected = np.asarray(reference.reference(**inputs))
    actual = kernel(**inputs)
    err = np.abs(actual - expected)
    print("absmax err:", err.max(), "scale:", np.abs(expected).max())
    print("rel fro:", np.linalg.norm(actual - expected) / np.linalg.norm(expected))


# revision 24
# speedup vs baseline: 121.0946x; 1.0038x over previous
"""Trainium2 Bass kernel for a dense transformer block (LN -> 16-head causal
attention -> residual -> LN -> FFN -> residual) on x:(2, 2048, 1024) fp32.

Sharding: 8 cores, zero collectives. Core c handles batch b=c//4, query chunk
a=c%4 (512 contiguous tokens). Every core recomputes full-sequence K/V for its
batch from a replicated (transposed) copy of x[b]; attention for the 512
queries runs against all 2048 keys with an additive causal mask supplied as
per-core input data, so the compiled program is identical across cores (SPMD).

Everything on-chip runs feature-on-partition ("T-layout"): LayerNorm statistics
are partition reductions done with ones-vector matmuls, the softmax denominator
comes from a ones column appended to V, and per-token stats are broadcast back
across partitions with gpsimd.partition_broadcast. Matmuls run in bf16 with
fp32 PSUM accumulation; both residual adds are carried in fp32.
"""

import numpy as np
import ml_dtypes

import concourse.bass as bass
import concourse.tile as tile
from concourse import bacc, mybir
from concourse import bass_utils
from concourse.bass import ts

P = 128
B, T, C = 2, 2048, 1024
H, D = 16, 64
FF = 4 * C
CC = C // P          # 8 feature chunks
TQ = 512             # queries per core
NSCH = T // P        # 16 key chunks
EPS = 1e-5
NEG = -30000.0
bf16 = ml_dtypes.bfloat16

f32 = mybir.dt.float32
bf = mybir.dt.bfloat16
AF = mybir.ActivationFunctionType
ALU = mybir.AluOpType


def _ln_T(nc, big1, chunked, spsum, x_tile, Tn, out_tile, g_sb, be_sb, eps11,
          ones1, x_is_f32):
    """LayerNorm over the feature dim with activations feature-on-partition.
    x_tile/out_tile: (P, CC, Tn). Stats via ones-matmul partition reduction,
    processed 512 tokens at a time. big1: bufs=1 pool; chunked: bufs>=2."""
    for tch in range(Tn // 512):
        xs = x_tile[:, :, ts(tch, 512)]
        ps = spsum.tile([1, 512], f32, tag="stat")
        if x_is_f32:
            for cc in range(CC):
                xbf = chunked.tile([P, 512], bf, tag="ln_xbf")
                nc.vector.tensor_copy(xbf, xs[:, cc, :])
                nc.tensor.matmul(ps, lhsT=ones1, rhs=xbf,
                                 start=(cc == 0), stop=(cc == CC - 1))
        else:
            for cc in range(CC):
                nc.tensor.matmul(ps, lhsT=ones1, rhs=xs[:, cc, :],
                                 start=(cc == 0), stop=(cc == CC - 1))
        pq = spsum.tile([1, 512], f32, tag="stat")
        for cc in range(CC):
            sq = chunked.tile([P, 512], bf, tag="ln_sq")
            nc.vector.tensor_mul(sq, xs[:, cc, :], xs[:, cc, :])
            nc.tensor.matmul(pq, lhsT=ones1, rhs=sq,
                             start=(cc == 0), stop=(cc == CC - 1))
        m = big1.tile([1, 512], f32, tag="ln_m")
        nc.vector.tensor_scalar_mul(m, ps, 1.0 / C)
        q = big1.tile([1, 512], f32, tag="ln_q")
        nc.vector.tensor_scalar_mul(q, pq, 1.0 / C)
        msq = big1.tile([1, 512], f32, tag="ln_msq")
        nc.vector.tensor_mul(msq, m, m)
        nc.vector.tensor_tensor(q, q, msq, ALU.subtract)  # q := var
        sd = big1.tile([1, 512], f32, tag="ln_sd")
        nc.scalar.activation(sd, q, AF.Sqrt, bias=eps11)
        a_t = big1.tile([1, 512], f32, tag="ln_at")
        nc.vector.reciprocal(a_t, sd)
        b_t = big1.tile([1, 512], f32, tag="ln_bt")
        nc.vector.tensor_mul(b_t, m, a_t)

        a_bc = chunked.tile([P, 512], f32, tag="ln_abc")
        nc.gpsimd.partition_broadcast(a_bc, a_t)
        b_bc = chunked.tile([P, 512], f32, tag="ln_bbc")
        nc.gpsimd.partition_broadcast(b_bc, b_t)
        for cc in range(CC):
            t1 = chunked.tile([P, 512], bf, tag="ln_t1")
            nc.vector.tensor_mul(t1, xs[:, cc, :], a_bc)
            nc.vector.tensor_tensor(t1, t1, b_bc, ALU.subtract)
            nc.vector.tensor_scalar(out_tile[:, cc, ts(tch, 512)], t1,
                                    scalar1=g_sb[:, cc:cc + 1],
                                    scalar2=be_sb[:, cc:cc + 1],
                                    op0=ALU.mult, op1=ALU.add)


FKV = 8 * 512 + 4 * H * 65          # AllGather payload per core (bf16 elems)


def _body(nc, tc, aps, use_ag, bounces):
    (xkvT, xqT, maskT, wq, wk, wv, wo, w1, w2,
     bo_t, b1_t, b2_t, g1_t, be1_t, g2_t, be2_t, outT) = aps

    import contextlib
    ctx = contextlib.ExitStack()
    with ctx:
        # pools that live for the whole kernel (small stuff + psum)
        consts = ctx.enter_context(tc.tile_pool(name="consts", bufs=1))
        small = ctx.enter_context(tc.tile_pool(name="small", bufs=2))
        ppool = ctx.enter_context(tc.tile_pool(name="ppool", bufs=5, space="PSUM"))
        opsum = ctx.enter_context(tc.tile_pool(name="opsum", bufs=1, space="PSUM"))
        spsum = ctx.enter_context(tc.tile_pool(name="spsum", bufs=2, space="PSUM"))

        ones1 = consts.tile([P, 1], bf)
        nc.vector.memset(ones1, 1.0)
        eps11 = consts.tile([1, 1], f32)
        nc.vector.memset(eps11, EPS)

        def load(pool, ap_dram, shape, dtype=f32, tag=None):
            t = pool.tile(list(shape), dtype, tag=tag or ap_dram.name)
            nc.sync.dma_start(t, ap_dram)
            return t

        bo_s = load(consts, bo_t, (P, CC))
        b1_s = load(consts, b1_t, (P, 32))
        b2_s = load(consts, b2_t, (P, CC))
        g1_s = load(consts, g1_t, (P, CC))
        be1_s = load(consts, be1_t, (P, CC))
        g2_s = load(consts, g2_t, (P, CC))
        be2_s = load(consts, be2_t, (P, CC))

        # ---- OT survives until the output projection (opened first: LIFO)
        opool = ctx.enter_context(tc.tile_pool(name="opool", bufs=1))

        # ---- KT/Vr/QT live from the projections to the end of attention
        kvq_ctx = contextlib.ExitStack()
        kvq = kvq_ctx.enter_context(tc.tile_pool(name="kvq", bufs=1))
        KT = kvq.tile([P, 8, T], bf)
        Vr = kvq.tile([P, NSCH, H, 65], bf)
        QT = kvq.tile([P, 8, TQ], bf)

        # ---- hkv/hq live until the end of the QKV projections
        with tc.tile_pool(name="hpool", bufs=1) as hpool:
            hq = hpool.tile([P, CC, TQ], bf)

            if not use_ag:
                hkv = hpool.tile([P, CC, T], bf)
                # LN1 over the full batch sequence (for K/V)
                with tc.tile_pool(name="p_ln1", bufs=1) as p1, \
                     tc.tile_pool(name="p_ln1b", bufs=1) as p1b, \
                     tc.tile_pool(name="p_ln1t", bufs=3) as p1t:
                    xkv_sb = load(p1, xkvT, (P, CC, T), bf)
                    _ln_T(nc, p1b, p1t, spsum, xkv_sb, T, hkv,
                          g1_s, be1_s, eps11, ones1, x_is_f32=False)

            # LN1 over the query slice
            with tc.tile_pool(name="p_ln1q", bufs=1) as p2, \
                 tc.tile_pool(name="p_ln1qb", bufs=1) as p2b, \
                 tc.tile_pool(name="p_ln1qt", bufs=2) as p2t:
                xq_sb = load(p2, xqT, (P, CC, TQ), f32, tag="xq_ln")
                _ln_T(nc, p2b, p2t, spsum, xq_sb, TQ, hq,
                      g1_s, be1_s, eps11, ones1, x_is_f32=True)

            with tc.tile_pool(name="p_w", bufs=1) as pw:
                wq_s = load(pw, wq, (P, CC, 8, P), bf)
                wk_s = load(pw, wk, (P, CC, 8, P), bf)
                wv_s = load(pw, wv, (P, CC, C), bf)

                if use_ag:
                    kv_in, kv_out = bounces
                    # own-chunk K^T (s = this core's 512 tokens)
                    KTo = pw.tile([P, CC, TQ], bf, tag="KTo")
                    for pair in range(8):
                        psum = ppool.tile([P, 512], f32, tag="mm")
                        for cc in range(CC):
                            nc.tensor.matmul(psum, lhsT=wk_s[:, cc, pair, :],
                                             rhs=hq[:, cc, :],
                                             start=(cc == 0), stop=(cc == CC - 1))
                        nc.vector.tensor_copy(KTo[:, pair, :], psum)
                    # own-chunk V rows (4 s-tiles) with ones column
                    Vro = pw.tile([P, 4, H, 65], bf, tag="Vro")
                    nc.vector.memset(Vro[:, :, :, 64:65], 1.0)
                    for st in range(4):
                        for half in range(2):
                            psum = ppool.tile([P, 512], f32, tag="mm")
                            for cc in range(CC):
                                nc.tensor.matmul(psum, lhsT=hq[:, cc, ts(st, P)],
                                                 rhs=wv_s[:, cc, ts(half, 512)],
                                                 start=(cc == 0), stop=(cc == CC - 1))
                            nc.vector.tensor_copy(
                                Vro[:, st, half * 8:(half + 1) * 8, 0:64],
                                psum.rearrange("p (h d) -> p h d", d=64))
                    # bounce out, AllGather within the 4-core batch group,
                    # then scatter the gathered chunks into KT / Vr
                    nc.sync.dma_start(
                        kv_in.ap()[:, 0:4096].rearrange("p (a b) -> p a b", a=CC),
                        KTo)
                    nc.sync.dma_start(
                        kv_in.ap()[:, 4096:FKV].rearrange(
                            "p (a h e) -> p a h e", a=4, h=H), Vro)
                    nc.gpsimd.collective_compute(
                        "AllGather",
                        mybir.AluOpType.bypass,
                        replica_groups=[[0, 1, 2, 3], [4, 5, 6, 7]],
                        ins=[kv_in.ap().opt()],
                        outs=[kv_out.ap().opt()],
                    )
                    for r in range(4):
                        blk = kv_out.ap()[r * P:(r + 1) * P, :]
                        nc.sync.dma_start(
                            KT[:, :, ts(r, 512)],
                            blk[:, 0:4096].rearrange("p (a b) -> p a b", a=CC))
                        nc.sync.dma_start(
                            Vr[:, 4 * r:4 * (r + 1), :, :],
                            blk[:, 4096:FKV].rearrange(
                                "p (a h e) -> p a h e", a=4, h=H))
                else:
                    for pair in range(8):
                        for sch in range(4):
                            psum = ppool.tile([P, 512], f32, tag="mm")
                            for cc in range(CC):
                                nc.tensor.matmul(psum, lhsT=wk_s[:, cc, pair, :],
                                                 rhs=hkv[:, cc, ts(sch, 512)],
                                                 start=(cc == 0), stop=(cc == CC - 1))
                            nc.vector.tensor_copy(KT[:, pair, ts(sch, 512)], psum)

                    nc.vector.memset(Vr[:, :, :, 64:65], 1.0)
                    for st in range(NSCH):
                        for half in range(2):
                            psum = ppool.tile([P, 512], f32, tag="mm")
                            for cc in range(CC):
                                nc.tensor.matmul(psum, lhsT=hkv[:, cc, ts(st, P)],
                                                 rhs=wv_s[:, cc, ts(half, 512)],
                                                 start=(cc == 0), stop=(cc == CC - 1))
                            nc.vector.tensor_copy(
                                Vr[:, st, half * 8:(half + 1) * 8, 0:64],
                                psum.rearrange("p (h d) -> p h d", d=64))

                for pair in range(8):
                    psum = ppool.tile([P, 512], f32, tag="mm")
                    for cc in range(CC):
                        nc.tensor.matmul(psum, lhsT=wq_s[:, cc, pair, :],
                                         rhs=hq[:, cc, :],
                                         start=(cc == 0), stop=(cc == CC - 1))
                    nc.vector.tensor_scalar_mul(QT[:, pair, :], psum,
                                                float(C) ** -0.5)

        # ---- attention
        OT = opool.tile([P, 8, TQ], bf)
        with tc.tile_pool(name="p_att", bufs=1) as pa, \
             tc.tile_pool(name="p_attt", bufs=4) as pat:
            mask_sb = load(pa, maskT, (P, NSCH, TQ), bf)
            for h in range(H):
                pair, half = h // 2, h % 2
                hp = slice(64 * half, 64 * half + 64)
                ops = opsum.tile([P, 512], f32, tag="av")
                for sch in range(NSCH):
                    sps = ppool.tile([P, 512], f32, tag="mm")
                    nc.tensor.matmul(sps, lhsT=KT[hp, pair, ts(sch, P)],
                                     rhs=QT[hp, pair, :], start=True, stop=True)
                    sm = pat.tile([P, TQ], bf, tag="sm")
                    nc.vector.tensor_tensor(sm, sps, mask_sb[:, sch, :], ALU.add)
                    e = pat.tile([P, TQ], bf, tag="e")
                    nc.scalar.activation(e, sm, AF.Exp)
                    nc.tensor.matmul(ops[0:65, :], lhsT=Vr[:, sch, h, :], rhs=e,
                                     start=(sch == 0), stop=(sch == NSCH - 1))
                zr = small.tile([1, TQ], f32, tag="zr")
                nc.vector.reciprocal(zr, ops[64:65, :])
                zb = pat.tile([64, TQ], f32, tag="zb")
                nc.gpsimd.partition_broadcast(zb, zr)
                nc.vector.tensor_mul(OT[hp, pair, :], ops[0:64, :], zb)
        kvq_ctx.close()

        # ---- output projection + bias + residual; LN2; FFN
        with tc.tile_pool(name="p_ffn", bufs=1) as pf, \
             tc.tile_pool(name="p_ffnt", bufs=2) as pft, \
             tc.tile_pool(name="p_wstream", bufs=3) as pws:
            xq_sb = load(pf, xqT, (P, CC, TQ), f32, tag="xq_res")
            wo_s = load(pf, wo, (P, CC, 8, P), bf)
            y1 = pf.tile([P, CC, TQ], f32)
            for mo in range(CC):
                psum = ppool.tile([P, 512], f32, tag="mm")
                for cc in range(CC):
                    nc.tensor.matmul(psum, lhsT=wo_s[:, cc, mo, :],
                                     rhs=OT[:, cc, :],
                                     start=(cc == 0), stop=(cc == CC - 1))
                t = pft.tile([P, TQ], f32, tag="res")
                nc.vector.tensor_scalar_add(t, psum, bo_s[:, mo:mo + 1])
                nc.vector.tensor_tensor(y1[:, mo, :], t, xq_sb[:, mo, :], ALU.add)

            h2 = pf.tile([P, CC, TQ], bf)
            with tc.tile_pool(name="p_ln2b", bufs=1) as pl2b:
                _ln_T(nc, pl2b, pft, spsum, y1, TQ, h2,
                      g2_s, be2_s, eps11, ones1, x_is_f32=True)

            zT = pf.tile([P, 32, TQ], bf)
            for m in range(32):
                w1b = pws.tile([P, CC, P], bf, tag="w1")
                nc.sync.dma_start(w1b, w1[m])
                psum = ppool.tile([P, 512], f32, tag="mm")
                for cc in range(CC):
                    nc.tensor.matmul(psum, lhsT=w1b[:, cc, :], rhs=h2[:, cc, :],
                                     start=(cc == 0), stop=(cc == CC - 1))
                nc.scalar.activation(zT[:, m, :], psum, AF.Relu,
                                     bias=b1_s[:, m:m + 1])

            for mo in range(CC):
                w2b = pws.tile([P, 32, P], bf, tag="w2")
                nc.sync.dma_start(w2b, w2[mo])
                psum = ppool.tile([P, 512], f32, tag="mm")
                for ff in range(32):
                    nc.tensor.matmul(psum, lhsT=w2b[:, ff, :], rhs=zT[:, ff, :],
                                     start=(ff == 0), stop=(ff == 31))
                t = pft.tile([P, TQ], f32, tag="res")
                nc.vector.tensor_scalar_add(t, psum, b2_s[:, mo:mo + 1])
                ot = pft.tile([P, TQ], f32, tag="ot")
                nc.vector.tensor_tensor(ot, t, y1[:, mo, :], ALU.add)
                nc.sync.dma_start(outT[:, mo, :], ot)


_NC_CACHE = {}
USE_AG = False


def build_nc(reps=1, use_ag=None):
    global _NC_CACHE
    if use_ag is None:
        use_ag = USE_AG
    key = (reps, use_ag)
    if key in _NC_CACHE:
        return _NC_CACHE[key]
    nc = bacc.Bacc("TRN2", target_bir_lowering=False, debug=False,
                   enable_asserts=False, num_devices=8)

    def dram(name, shape, dtype, kind="ExternalInput"):
        return nc.dram_tensor(name, shape, dtype, kind=kind).ap()

    aps = (
        dram("xkvT", (P, CC, T), bf) if not use_ag else None,
        dram("xqT", (P, CC, TQ), f32),
        dram("maskT", (P, NSCH, TQ), bf),
        dram("wq", (P, CC, 8, P), bf),
        dram("wk", (P, CC, 8, P), bf),
        dram("wv", (P, CC, C), bf),
        dram("wo", (P, CC, 8, P), bf),
        dram("w1", (32, P, CC, P), bf),
        dram("w2", (CC, P, 32, P), bf),
        dram("bo_t", (P, CC), f32),
        dram("b1_t", (P, 32), f32),
        dram("b2_t", (P, CC), f32),
        dram("g1_t", (P, CC), f32),
        dram("be1_t", (P, CC), f32),
        dram("g2_t", (P, CC), f32),
        dram("be2_t", (P, CC), f32),
        dram("outT", (P, CC, TQ), f32, kind="ExternalOutput"),
    )
    bounces = None
    if use_ag:
        bounces = []
        for i in range(reps):
            kv_in = nc.dram_tensor(f"kv_in{i}", (P, FKV), bf)
            kv_out = nc.dram_tensor(f"kv_out{i}", (4 * P, FKV), bf)
            bounces.append((kv_in, kv_out))
    with tile.TileContext(nc) as tc:
        for i in range(reps):
            _body(nc, tc, aps, use_ag, bounces[i] if use_ag else None)
    nc.compile()
    _NC_CACHE[key] = nc
    return nc


def _tile_lhst(w):  # (C, C) -> (P, cc, pair/mo, 128)
    return np.ascontiguousarray(
        w.reshape(CC, P, 8, P).transpose(1, 0, 2, 3)).astype(bf16)


def make_in_maps(inputs, use_ag=None):
    """Build the 8 per-core input dicts from the full problem inputs."""
    if use_ag is None:
        use_ag = USE_AG
    x = np.asarray(inputs["x"], np.float32)
    Wq = np.asarray(inputs["Wq"], np.float32)
    Wk = np.asarray(inputs["Wk"], np.float32)
    Wv = np.asarray(inputs["Wv"], np.float32)
    Wo = np.asarray(inputs["Wo"], np.float32)
    W1 = np.asarray(inputs["W1"], np.float32)
    W2 = np.asarray(inputs["W2"], np.float32)

    wq_flat = np.ascontiguousarray(Wq.transpose(1, 0, 2)).reshape(C, C)
    wk_flat = np.ascontiguousarray(Wk.transpose(1, 0, 2)).reshape(C, C)
    wv_flat = np.ascontiguousarray(Wv.transpose(1, 0, 2)).reshape(C, C)

    shared = {
        "wq": _tile_lhst(wq_flat),
        "wk": _tile_lhst(wk_flat),
        "wv": np.ascontiguousarray(
            wv_flat.reshape(CC, P, C).transpose(1, 0, 2)).astype(bf16),
        "wo": _tile_lhst(Wo),
        "w1": np.ascontiguousarray(
            W1.reshape(CC, P, 32, P).transpose(2, 1, 0, 3)).astype(bf16),
        "w2": np.ascontiguousarray(
            W2.reshape(32, P, CC, P).transpose(2, 1, 0, 3)).astype(bf16),
        "bo_t": np.ascontiguousarray(
            np.asarray(inputs["bo"], np.float32).reshape(CC, P).T),
        "b1_t": np.ascontiguousarray(
            np.asarray(inputs["b1"], np.float32).reshape(32, P).T),
        "b2_t": np.ascontiguousarray(
            np.asarray(inputs["b2"], np.float32).reshape(CC, P).T),
        "g1_t": np.ascontiguousarray(
            np.asarray(inputs["g1"], np.float32).reshape(CC, P).T),
        "be1_t": np.ascontiguousarray(
            np.asarray(inputs["be1"], np.float32).reshape(CC, P).T),
        "g2_t": np.ascontiguousarray(
            np.asarray(inputs["g2"], np.float32).reshape(CC, P).T),
        "be2_t": np.ascontiguousarray(
            np.asarray(inputs["be2"], np.float32).reshape(CC, P).T),
    }

    s_idx = np.arange(T)
    in_maps = []
    for c in range(8):
        b, a = c // 4, c % 4
        q0 = TQ * a
        xbT = np.ascontiguousarray(x[b].T)                       # (C, T)
        xkvT = xbT.reshape(CC, P, T).transpose(1, 0, 2).astype(bf16)
        xqT = np.ascontiguousarray(
            xbT[:, q0:q0 + TQ].reshape(CC, P, TQ).transpose(1, 0, 2))
        mask = np.where(s_idx[:, None] <= (q0 + np.arange(TQ))[None, :],
                        np.float32(0.0), np.float32(NEG))
        maskT = mask.reshape(NSCH, P, TQ).transpose(1, 0, 2).astype(bf16)
        m = {
            "xqT": xqT.astype(np.float32),
            "maskT": np.ascontiguousarray(maskT),
            **shared,
        }
        if not use_ag:
            m["xkvT"] = np.ascontiguousarray(xkvT)
        in_maps.append(m)
    return in_maps


def assemble_output(core_outs):
    """core_outs: list of 8 dicts with 'outT' (P, CC, TQ) fp32."""
    out = np.zeros((B, T, C), np.float32)
    for c in range(8):
        b, a = c // 4, c % 4
        y2 = core_outs[c]["outT"].transpose(1, 0, 2).reshape(C, TQ)  # (C, TQ)
        out[b, TQ * a:TQ * (a + 1), :] = y2.T
    return out


def kernel(**inputs) -> np.ndarray:
    nc = build_nc()
    in_maps = make_in_maps(inputs)
    res = bass_utils.run_bass_kernel_spmd(nc, in_maps, core_ids=list(range(8)))
    return assemble_output(res.results)


if __name__ == "__main__":
    import reference
    inputs = {k: np.asarray(v) for k, v in reference.setup_inputs().items()}
    expected = np.asarray(reference.reference(**inputs))
    actual = kernel(**inputs)
    err = np.abs(actual - expected)
    print("absmax err:", err.max(), "scale:", np.abs(expected).max())
    print("rel fro:", np.linalg.norm(actual - expected) / np.linalg.norm(expected))
